# revision 2
# baseline (speedup 1.0000x reference)
"""CDist kernel for Trainium2 (8 NeuronCores, SPMD data-parallel over x rows).

out[i, j] = sqrt(sum_d (x[i,d] - y[j,d])^2),  x: [2048, 64], y: [2048, 64].

Strategy: shard x rows 8 ways (256 rows/core), replicate y. Each core computes
its [256, 2048] tile with a single augmented matmul per output block:
  d2[i,j] = ||x_i||^2 + ||y_j||^2 - 2 x_i.y_j
          = dot(xa_i, ya_j)   with K = 64 + 2 = 66
  xa = [x_i, ||x_i||^2, 1],  ya = [-2 y_j, 1, ||y_j||^2]
then sqrt on the scalar (ACT) engine straight out of PSUM, and DMA to DRAM.
"""

import numpy as np

N = 2048
D = 64
N_CORES = 8
ROWS_PER_CORE = N // N_CORES  # 256

K_AUG = D + 2  # 66
M_TILE = 128
N_TILE = 512
N_MTILES = ROWS_PER_CORE // M_TILE  # 2
N_NTILES = N // N_TILE  # 4
N_YTILES = N // 128  # 16

_compiled = {}


def _build_nc(n_iters=1):
    from contextlib import ExitStack

    import concourse.bacc as bacc
    import concourse.tile as tile
    from concourse import mybir
    from concourse.masks import make_identity

    f32 = mybir.dt.float32
    nc = bacc.Bacc("TRN2", target_bir_lowering=False, debug=False,
                   num_devices=N_CORES)
    xs = nc.dram_tensor("xs", [ROWS_PER_CORE, D], f32, kind="ExternalInput")
    y = nc.dram_tensor("y", [N, D], f32, kind="ExternalInput")
    out = nc.dram_tensor("out", [ROWS_PER_CORE, N], f32, kind="ExternalOutput")

    with tile.TileContext(nc) as tc, ExitStack() as ctx:
        singles = ctx.enter_context(tc.tile_pool(name="singles", bufs=1))
        mats = ctx.enter_context(tc.tile_pool(name="mats", bufs=1))
        loads = ctx.enter_context(tc.tile_pool(name="loads", bufs=4))
        augs = ctx.enter_context(tc.tile_pool(name="augs", bufs=4))
        scratch = ctx.enter_context(tc.tile_pool(name="scratch", bufs=4))
        tp_psum = ctx.enter_context(tc.tile_pool(name="tp_psum", bufs=2, space="PSUM"))
        mm_psum = ctx.enter_context(tc.tile_pool(name="mm_psum", bufs=4, space="PSUM"))
        outs = ctx.enter_context(tc.tile_pool(name="outs", bufs=4))

        identity = singles.tile([128, 128], f32)
        make_identity(nc, identity)

        for _it in range(n_iters):
            xaT = mats.tile([K_AUG, ROWS_PER_CORE], f32, tag="xaT")  # [66, 256]
            yaT = mats.tile([K_AUG, N], f32, tag="yaT")  # [66, 2048]

            # Build yaT: rows 0..63 = -2*y^T, row 64 = 1, row 65 = ||y_j||^2
            for j in range(N_YTILES):
                yt = loads.tile([128, D], f32, tag="ld")
                nc.sync.dma_start(out=yt, in_=y[j * 128:(j + 1) * 128, :])
                aug = augs.tile([128, K_AUG], f32, tag="aug")
                sq = scratch.tile([128, D], f32, tag="sq")
                # sq = y^2 (discarded); accum_out = row-sum(y^2) -> aug col 65
                nc.scalar.activation(out=sq, in_=yt,
                                     func=mybir.ActivationFunctionType.Square,
                                     accum_out=aug[:, D + 1:D + 2])
                nc.scalar.mul(out=aug[:, 0:D], in_=yt, mul=-2.0)
                nc.vector.memset(aug[:, D:D + 1], 1.0)
                ps = tp_psum.tile([K_AUG, 128], f32, tag="tp")
                nc.tensor.transpose(ps, aug, identity)
                nc.vector.tensor_copy(out=yaT[:, j * 128:(j + 1) * 128], in_=ps)

            # Build xaT: rows 0..63 = x^T, row 64 = ||x_i||^2, row 65 = 1
            for m in range(N_MTILES):
                xt = loads.tile([128, D], f32, tag="ld")
                nc.sync.dma_start(out=xt, in_=xs[m * 128:(m + 1) * 128, :])
                aug = augs.tile([128, K_AUG], f32, tag="aug")
                sq = scratch.tile([128, D], f32, tag="sq")
                nc.scalar.activation(out=sq, in_=xt,
                                     func=mybir.ActivationFunctionType.Square,
                                     accum_out=aug[:, D:D + 1])
                nc.scalar.copy(out=aug[:, 0:D], in_=xt)
                nc.vector.memset(aug[:, D + 1:D + 2], 1.0)
                ps = tp_psum.tile([K_AUG, 128], f32, tag="tp")
                nc.tensor.transpose(ps, aug, identity)
                nc.vector.tensor_copy(out=xaT[:, m * 128:(m + 1) * 128], in_=ps)

            # Distance blocks: matmul -> sqrt -> DMA out
            for m in range(N_MTILES):
                for n in range(N_NTILES):
                    ps = mm_psum.tile([M_TILE, N_TILE], f32, tag="mm")
                    nc.tensor.matmul(ps,
                                     lhsT=xaT[:, m * M_TILE:(m + 1) * M_TILE],
                                     rhs=yaT[:, n * N_TILE:(n + 1) * N_TILE],
                                     start=True, stop=True)
                    ot = outs.tile([M_TILE, N_TILE], f32, tag="ot")
                    nc.scalar.activation(out=ot, in_=ps,
                                         func=mybir.ActivationFunctionType.Sqrt)
                    nc.sync.dma_start(
                        out=out[m * M_TILE:(m + 1) * M_TILE,
                                n * N_TILE:(n + 1) * N_TILE],
                        in_=ot)

    nc.compile()
    return nc


def _get_nc():
    if "nc" not in _compiled:
        _compiled["nc"] = _build_nc()
    return _compiled["nc"]


def kernel(x, y, _trace=False, **_ignored):
    from concourse.bass_utils import run_bass_kernel_spmd

    x = np.ascontiguousarray(np.asarray(x), dtype=np.float32)
    y = np.ascontiguousarray(np.asarray(y), dtype=np.float32)
    assert x.shape == (N, D) and y.shape == (N, D)

    nc = _get_nc()
    in_maps = [
        {"xs": x[c * ROWS_PER_CORE:(c + 1) * ROWS_PER_CORE, :], "y": y}
        for c in range(N_CORES)
    ]
    res = run_bass_kernel_spmd(nc, in_maps, core_ids=list(range(N_CORES)),
                               trace=_trace)
    full = np.concatenate([res.results[c]["out"] for c in range(N_CORES)], axis=0)
    if _trace:
        return full, res
    return full


# revision 29
# speedup vs baseline: 12.1809x; 12.1809x over previous
"""CDist kernel for Trainium2 (8 NeuronCores, SPMD data-parallel over x rows).

out[i, j] = sqrt(sum_d (x[i,d] - y[j,d])^2),  x: [2048, 64], y: [2048, 64].

Sharding: x rows split 8 ways (256 rows/core), y replicated. Each core
computes its [256, 2048] tile via the expansion
  d2[i,j] = ||x_i||^2 + ||y_j||^2 - 2 x_i.y_j
as one K=65 matmul per output block plus a per-partition bias in the sqrt:
  lhsT = xaT [65, 128]:  rows 0..63 = -2*x^T chunk, row 64 = ones
  rhs  = yaT [65, 512]:  rows 0..63 = y^T chunk,    row 64 = ||y_j||^2
  psum = -2 x.y + y2_j ;  out = ACT sqrt(psum + x2_i)  (bias per partition)
Transposes are PE-based (fp32 has no DMA transpose). The y2 row of yaT is
assembled with a single [128,16] PE transpose + one PSUM->SBUF DMA flatten.
"""

import os

import numpy as np

# Persistent XLA/NEFF compile cache so repeated runs skip recompilation.
os.environ.setdefault("JAX_COMPILATION_CACHE_DIR", "/tmp/jax_comp_cache")

N = 2048
D = 64
N_CORES = 8
ROWS_PER_CORE = N // N_CORES  # 256

K_AUG = D + 1  # 65
M_TILE = 128
N_TILE = 512
N_MTILES = ROWS_PER_CORE // M_TILE  # 2
N_NTILES = N // N_TILE  # 4
N_YTILES = N // 128  # 16

# main matmul in float32r: 1 cycle/row on PE vs 4 for fp32 (verified on HW
# for accuracy by test.py before this was enabled)
MM_F32R = True

_cache = {}


def _build_nc(n_iters=1, mm_f32r=MM_F32R, OUT_W=1024, _skip=()):
    # _skip: debug/profiling knob for timeline-sim bisection
    # ("out_dma", "mm", "sqrt", "ybuild")
    from contextlib import ExitStack

    import concourse.bacc as bacc
    import concourse.tile as tile
    from concourse import mybir
    from concourse.masks import make_identity

    f32 = mybir.dt.float32
    f32r = mybir.dt.float32r
    Alu = mybir.AluOpType
    Act = mybir.ActivationFunctionType

    nc = bacc.Bacc("TRN2", target_bir_lowering=False, debug=False,
                   num_devices=N_CORES)
    xs = nc.dram_tensor("xs", [ROWS_PER_CORE, D], f32, kind="ExternalInput")
    y = nc.dram_tensor("y", [N, D], f32, kind="ExternalInput")
    out = nc.dram_tensor("out", [ROWS_PER_CORE, N], f32, kind="ExternalOutput")

    # fp32r matmul inputs must be *rounded* to fp32r by their producer
    # instruction, so the staged transposed matrices are declared f32r and
    # the PSUM->SBUF copies do the rounding.
    mmdt = f32r if mm_f32r else f32

    with tile.TileContext(nc) as tc, ExitStack() as ctx:
        singles = ctx.enter_context(tc.tile_pool(name="singles", bufs=1))
        mats = ctx.enter_context(tc.tile_pool(name="mats", bufs=1))
        xloads = ctx.enter_context(tc.tile_pool(name="xloads", bufs=2))
        yloads = ctx.enter_context(tc.tile_pool(name="yloads", bufs=3))
        augs = ctx.enter_context(tc.tile_pool(name="augs", bufs=2))
        scratch = ctx.enter_context(tc.tile_pool(name="scratch", bufs=4))
        tp_psum = ctx.enter_context(tc.tile_pool(name="tp_psum", bufs=3, space="PSUM"))
        mm_psum = ctx.enter_context(tc.tile_pool(name="mm_psum", bufs=2, space="PSUM"))
        outs = ctx.enter_context(tc.tile_pool(name="outs", bufs=4))

        identity = singles.tile([128, 128], f32)
        dummy = singles.tile([128, 1], f32)

        for _it in range(n_iters):
            xaT = mats.tile([K_AUG, ROWS_PER_CORE], mmdt, tag="xaT")  # [65, 256]
            yaT = mats.tile([K_AUG, N], mmdt, tag="yaT")  # [65, 2048]
            x2cols = mats.tile([128, N_MTILES], f32, tag="x2")  # sqrt bias

            if _it == 0:
                # Identity (for PE transposes) + sqrt ACT-table preload go
                # FIRST: everything downstream waits on them, and they only
                # cost ~0.5us on otherwise-idle engines.
                make_identity(nc, identity)
                nc.vector.memset(dummy, 1.0)
                nc.scalar.activation(out=dummy, in_=dummy, func=Act.Sqrt)

            # Kick off ALL input DMAs (HWDGE; descriptor gen in HW).
            # x lands in cols 0..63 of a [128, 2, 65] staging tile; y groups
            # land in cols 0..63 of [128, g, 65] tiles. Col 64 is filled with
            # the squared row norm by DVE, so ONE PE transpose per tile
            # yields both the transposed data block and the norm row.
            xt = xloads.tile([128, N_MTILES, K_AUG], f32, tag="xld")
            nc.sync.dma_start(
                out=xt[:, :, 0:D],
                in_=xs[:, :].rearrange("(t p) d -> p t d", p=128))
            y_groups = [(0, 2), (256, 2), (512, 4), (1024, 4), (1536, 4)]
            yts = []
            for gi, (row0, g) in enumerate(y_groups):
                yt = yloads.tile([128, g, K_AUG], f32, tag=f"yld{gi}")
                nc.sync.dma_start(
                    out=yt[:, :, 0:D],
                    in_=y[row0:row0 + g * 128, :].rearrange(
                        "(t p) d -> p t d", p=128))
                yts.append(yt)

            # ---- x side: xaT rows 0..63 = x^T (raw), row 64 = ones.
            # The -2 factor is applied by the sqrt activation's scale, and
            # the y-norm row carries -||y||^2/2, so no scaling pass on x is
            # needed: the PE transposes read the load tile directly.
            for m in range(N_MTILES):
                sq = scratch.tile([128, D], f32, tag="sq")
                nc.vector.tensor_mul(out=sq, in0=xt[:, m, 0:D],
                                     in1=xt[:, m, 0:D])
                nc.vector.reduce_sum(out=x2cols[:, m:m + 1], in_=sq,
                                     axis=mybir.AxisListType.X)
                nc.vector.memset(xt[:, m, D:D + 1], 1.0)
                ps = tp_psum.tile([K_AUG, 128], f32, tag="tp",
                                  name=f"tpx_{_it}_{m}")
                nc.tensor.transpose(ps, xt[:, m, :], identity)
                nc.vector.tensor_copy(out=xaT[:, m * 128:(m + 1) * 128], in_=ps)

            # ---- y side, one 512-col unit at a time, matmuls interleaved
            # so the output DMA starts as soon as the first unit is built.
            # Output stores are grouped into [128, OUT_W] tiles to amortize
            # the per-op HWDGE ring cost.
            mm_tiles = {}
            for u, (col0, g) in enumerate(y_groups):
                yt = yts[u]
                U = g * 128
                ps = tp_psum.tile([K_AUG, U], f32, tag="tp",
                                  name=f"tp_{_it}_{u}")
                for t in range(g):
                    # square + row-sum scaled by -1/2 -> col 64
                    # (the -||y||^2/2 norm row)
                    sq = scratch.tile([128, D], f32, tag="sq")
                    nc.vector.scalar_tensor_tensor(
                        out=sq, in0=yt[:, t, 0:D], scalar=-0.5,
                        in1=yt[:, t, 0:D],
                        op0=Alu.mult, op1=Alu.mult)
                    nc.vector.reduce_sum(out=yt[:, t, D:D + 1], in_=sq,
                                         axis=mybir.AxisListType.X)
                    nc.tensor.transpose(ps[:, t * 128:(t + 1) * 128],
                                        yt[:, t, :], identity)
                # one wide PSUM->SBUF copy per unit
                nc.vector.tensor_copy(out=yaT[:, col0:col0 + U], in_=ps)

                # distance blocks: matmuls fill an OUT_W-wide 2-bank PSUM
                # tile; one wide sqrt (scale=-2, bias=||x||^2) writes SBUF,
                # then the store fires immediately.
                if "mm" in _skip:
                    continue
                ow = OUT_W
                base = (col0 // ow) * ow
                for m in range(N_MTILES):
                    key = (m, base)
                    if key not in mm_tiles:
                        mm_tiles[key] = mm_psum.tile(
                            [M_TILE, ow], f32, tag="mm",
                            name=f"mm_{_it}_{m}_{base}")
                    psm = mm_tiles[key]
                    nc.tensor.matmul(
                        psm[:, col0 - base:col0 - base + U],
                        lhsT=xaT[:, m * M_TILE:(m + 1) * M_TILE],
                        rhs=yaT[:, col0:col0 + U],
                        start=True, stop=True)
                    if col0 - base + U != ow:
                        continue
                    ot = outs.tile([M_TILE, ow], f32, tag="ot",
                                   name=f"ot_{_it}_{m}_{base}")
                    if "sqrt" in _skip:
                        nc.vector.tensor_copy(out=ot, in_=psm)
                    else:
                        nc.scalar.activation(out=ot, in_=psm, func=Act.Sqrt,
                                             bias=x2cols[:, m:m + 1],
                                             scale=-2.0)
                    if "out_dma" in _skip:
                        continue
                    # alternate the two HWDGE rings (SP + ACT paths)
                    dma_eng = nc.sync if m == 0 else nc.scalar
                    dma_eng.dma_start(
                        out=out[m * M_TILE:(m + 1) * M_TILE,
                                base:base + ow],
                        in_=ot)

    nc.compile()
    return nc


def _make_runner(nc):
    """Cached jitted SPMD executor (mirrors bass2jax.run_bass_via_pjrt, but
    reuses one jax.jit wrapper so the NEFF is not re-loaded per call)."""
    import jax
    from jax.experimental.shard_map import shard_map
    from jax.sharding import Mesh, PartitionSpec

    from concourse import bass2jax, mybir

    bass2jax.install_neuronx_cc_hook()
    assert nc.dbg_addr is None

    partition_name = (nc.partition_id_tensor.name
                      if nc.partition_id_tensor else None)
    in_names, out_names, out_avals, zero_shapes = [], [], [], []
    for alloc in nc.m.functions[0].allocations:
        if not isinstance(alloc, mybir.MemoryLocationSet):
            continue
        name = alloc.memorylocations[0].name
        if alloc.kind == "ExternalInput":
            if name != partition_name:
                in_names.append(name)
        elif alloc.kind == "ExternalOutput":
            shape = tuple(alloc.tensor_shape)
            dtype = mybir.dt.np(alloc.dtype)
            out_names.append(name)
            out_avals.append(jax.core.ShapedArray(shape, dtype))
            zero_shapes.append((shape, dtype))
    n_params = len(in_names)
    n_outs = len(out_names)
    all_in_names = list(in_names + out_names)
    if partition_name is not None:
        all_in_names.append(partition_name)
    all_in_names = tuple(all_in_names)
    donate = tuple(range(n_params, n_params + n_outs))

    def _body(*args):
        operands = list(args)
        if partition_name is not None:
            operands.append(bass2jax.partition_id_tensor())
        outs = bass2jax._bass_exec_p.bind(
            *operands,
            out_avals=tuple(out_avals),
            in_names=all_in_names,
            out_names=tuple(out_names),
            lowering_input_output_aliases=(),
            sim_require_finite=True,
            sim_require_nnan=True,
            nc=nc,
        )
        return tuple(outs)

    devices = jax.devices()[:N_CORES]
    mesh = Mesh(np.asarray(devices), ("core",))
    sharded = jax.jit(
        shard_map(_body, mesh=mesh,
                  in_specs=(PartitionSpec("core"),) * (n_params + n_outs),
                  out_specs=(PartitionSpec("core"),) * n_outs,
                  check_rep=False),
        donate_argnums=donate, keep_unused=True)

    def run(in_maps):
        concat_in = [
            np.concatenate([np.asarray(m[name]) for m in in_maps], axis=0)
            for name in in_names
        ]
        concat_zeros = [
            np.zeros((N_CORES * s[0], *s[1:]), dt) for s, dt in zero_shapes
        ]
        out_arrs = sharded(*concat_in, *concat_zeros)
        return [
            {name: np.asarray(out_arrs[i]).reshape(N_CORES, *zero_shapes[i][0])[c]
             for i, name in enumerate(out_names)}
            for c in range(N_CORES)
        ]

    return run


def _get_runner():
    if "run" not in _cache:
        _cache["run"] = _make_runner(_build_nc())
    return _cache["run"]


def kernel(x, y, **_ignored):
    x = np.ascontiguousarray(np.asarray(x), dtype=np.float32)
    y = np.ascontiguousarray(np.asarray(y), dtype=np.float32)
    assert x.shape == (N, D) and y.shape == (N, D)

    run = _get_runner()
    in_maps = [
        {"xs": x[c * ROWS_PER_CORE:(c + 1) * ROWS_PER_CORE, :], "y": y}
        for c in range(N_CORES)
    ]
    results = run(in_maps)
    return np.concatenate([results[c]["out"] for c in range(N_CORES)], axis=0)


# revision 31
# speedup vs baseline: 151.5333x; 12.4403x over previous
"""CDist kernel for Trainium2 (8 NeuronCores, SPMD data-parallel over x rows).

out[i, j] = sqrt(sum_d (x[i,d] - y[j,d])^2),  x: [2048, 64], y: [2048, 64].

Sharding: x rows split 8 ways (256 rows/core), y replicated. Each core
computes its [256, 2048] tile via the expansion
  d2[i,j] = ||x_i||^2 + ||y_j||^2 - 2 x_i.y_j
as one K=65 matmul per output block plus a per-partition bias in the sqrt:
  lhsT = xaT [65, 128]: rows 0..63 = x^T chunk,  row 64 = ones
  rhs  = yaT [65, 512]: rows 0..63 = y^T chunk,  row 64 = -||y_j||^2/2
  psum = x.y - y2_j/2 ;  out = ACT sqrt(-2*psum + x2_i)  (bias, scale=-2)
Transposes are PE-based (fp32 has no DMA transpose); the norm row rides
each [128, 65] tile transpose, so no extra data movement assembles it.
The kernel streams y in five groups, interleaving transpose -> matmul ->
sqrt -> store per 256/512-column unit so the output DMA (the roofline
term: 2 MB/core) starts as early as possible.
"""

import os

import numpy as np

# Persistent XLA/NEFF compile cache so repeated runs skip recompilation.
os.environ.setdefault("JAX_COMPILATION_CACHE_DIR", "/tmp/jax_comp_cache")

N = 2048
D = 64
N_CORES = 8
ROWS_PER_CORE = N // N_CORES  # 256

K_AUG = D + 1  # 65
M_TILE = 128
N_MTILES = ROWS_PER_CORE // M_TILE  # 2

# float32r matmul is 1 cycle/row on PE vs 4 for fp32, but HW-measured
# rel err is 1.5e-4 vs 7e-6 for fp32 — marginal against a strict absmax
# gate, so the default stays fp32 (the kernel is DMA-bound anyway).
MM_F32R = False

_cache = {}


def _build_nc(n_iters=1, mm_f32r=MM_F32R, OUT_W=1024, _skip=()):
    # _skip: debug/profiling knob for timeline-sim bisection
    # ("out_dma", "mm", "sqrt", "ybuild")
    from contextlib import ExitStack

    import concourse.bacc as bacc
    import concourse.tile as tile
    from concourse import mybir
    from concourse.masks import make_identity

    f32 = mybir.dt.float32
    f32r = mybir.dt.float32r
    Alu = mybir.AluOpType
    Act = mybir.ActivationFunctionType

    nc = bacc.Bacc("TRN2", target_bir_lowering=False, debug=False,
                   num_devices=N_CORES)
    xs = nc.dram_tensor("xs", [ROWS_PER_CORE, D], f32, kind="ExternalInput")
    y = nc.dram_tensor("y", [N, D], f32, kind="ExternalInput")
    out = nc.dram_tensor("out", [ROWS_PER_CORE, N], f32, kind="ExternalOutput")

    # fp32r matmul inputs must be *rounded* to fp32r by their producer
    # instruction, so the staged transposed matrices are declared f32r and
    # the PSUM->SBUF copies do the rounding.
    mmdt = f32r if mm_f32r else f32

    with tile.TileContext(nc) as tc, ExitStack() as ctx:
        singles = ctx.enter_context(tc.tile_pool(name="singles", bufs=1))
        mats = ctx.enter_context(tc.tile_pool(name="mats", bufs=1))
        xloads = ctx.enter_context(tc.tile_pool(name="xloads", bufs=2))
        yloads = ctx.enter_context(tc.tile_pool(name="yloads", bufs=3))
        scratch = ctx.enter_context(tc.tile_pool(name="scratch", bufs=4))
        tp_psum = ctx.enter_context(tc.tile_pool(name="tp_psum", bufs=3, space="PSUM"))
        mm_psum = ctx.enter_context(tc.tile_pool(name="mm_psum", bufs=2, space="PSUM"))
        outs = ctx.enter_context(tc.tile_pool(name="outs", bufs=4))

        identity = singles.tile([128, 128], f32)
        dummy = singles.tile([128, 1], f32)

        for _it in range(n_iters):
            xaT = mats.tile([K_AUG, ROWS_PER_CORE], mmdt, tag="xaT")  # [65, 256]
            yaT = mats.tile([K_AUG, N], mmdt, tag="yaT")  # [65, 2048]
            x2cols = mats.tile([128, N_MTILES], f32, tag="x2")  # sqrt bias

            if _it == 0:
                # Identity (for PE transposes) + sqrt ACT-table preload go
                # FIRST: everything downstream waits on them, and they only
                # cost ~0.5us on otherwise-idle engines.
                make_identity(nc, identity)
                nc.vector.memset(dummy, 1.0)
                nc.scalar.activation(out=dummy, in_=dummy, func=Act.Sqrt)

            # Kick off ALL input DMAs (HWDGE; descriptor gen in HW).
            # x lands in cols 0..63 of a [128, 2, 65] staging tile; y groups
            # land in cols 0..63 of [128, g, 65] tiles. Col 64 is filled with
            # the squared row norm by DVE, so ONE PE transpose per tile
            # yields both the transposed data block and the norm row.
            xt = xloads.tile([128, N_MTILES, K_AUG], f32, tag="xld")
            nc.sync.dma_start(
                out=xt[:, :, 0:D],
                in_=xs[:, :].rearrange("(t p) d -> p t d", p=128))
            y_groups = [(0, 2), (256, 2), (512, 4), (1024, 4), (1536, 4)]
            yts = []
            for gi, (row0, g) in enumerate(y_groups):
                yt = yloads.tile([128, g, K_AUG], f32, tag=f"yld{gi}")
                nc.sync.dma_start(
                    out=yt[:, :, 0:D],
                    in_=y[row0:row0 + g * 128, :].rearrange(
                        "(t p) d -> p t d", p=128))
                yts.append(yt)

            # ---- x side: xaT rows 0..63 = x^T (raw), row 64 = ones.
            # The -2 factor is applied by the sqrt activation's scale, and
            # the y-norm row carries -||y||^2/2, so no scaling pass on x is
            # needed: the PE transposes read the load tile directly.
            for m in range(N_MTILES):
                sq = scratch.tile([128, D], f32, tag="sq")
                nc.vector.tensor_mul(out=sq, in0=xt[:, m, 0:D],
                                     in1=xt[:, m, 0:D])
                nc.vector.reduce_sum(out=x2cols[:, m:m + 1], in_=sq,
                                     axis=mybir.AxisListType.X)
                nc.vector.memset(xt[:, m, D:D + 1], 1.0)
                ps = tp_psum.tile([K_AUG, 128], f32, tag="tp",
                                  name=f"tpx_{_it}_{m}")
                nc.tensor.transpose(ps, xt[:, m, :], identity)
                nc.vector.tensor_copy(out=xaT[:, m * 128:(m + 1) * 128], in_=ps)

            # ---- y side, one 512-col unit at a time, matmuls interleaved
            # so the output DMA starts as soon as the first unit is built.
            # Output stores are grouped into [128, OUT_W] tiles to amortize
            # the per-op HWDGE ring cost.
            mm_tiles = {}
            for u, (col0, g) in enumerate(y_groups):
                yt = yts[u]
                U = g * 128
                ps = tp_psum.tile([K_AUG, U], f32, tag="tp",
                                  name=f"tp_{_it}_{u}")
                for t in range(g):
                    # square + row-sum scaled by -1/2 -> col 64
                    # (the -||y||^2/2 norm row)
                    sq = scratch.tile([128, D], f32, tag="sq")
                    nc.vector.scalar_tensor_tensor(
                        out=sq, in0=yt[:, t, 0:D], scalar=-0.5,
                        in1=yt[:, t, 0:D],
                        op0=Alu.mult, op1=Alu.mult)
                    nc.vector.reduce_sum(out=yt[:, t, D:D + 1], in_=sq,
                                         axis=mybir.AxisListType.X)
                    nc.tensor.transpose(ps[:, t * 128:(t + 1) * 128],
                                        yt[:, t, :], identity)
                # one wide PSUM->SBUF copy per unit
                nc.vector.tensor_copy(out=yaT[:, col0:col0 + U], in_=ps)

                # distance blocks: matmuls fill an OUT_W-wide 2-bank PSUM
                # tile; one wide sqrt (scale=-2, bias=||x||^2) writes SBUF,
                # then the store fires immediately.
                if "mm" in _skip:
                    continue
                ow = OUT_W
                base = (col0 // ow) * ow
                for m in range(N_MTILES):
                    key = (m, base)
                    if key not in mm_tiles:
                        mm_tiles[key] = mm_psum.tile(
                            [M_TILE, ow], f32, tag="mm",
                            name=f"mm_{_it}_{m}_{base}")
                    psm = mm_tiles[key]
                    nc.tensor.matmul(
                        psm[:, col0 - base:col0 - base + U],
                        lhsT=xaT[:, m * M_TILE:(m + 1) * M_TILE],
                        rhs=yaT[:, col0:col0 + U],
                        start=True, stop=True)
                    if col0 - base + U != ow:
                        continue
                    ot = outs.tile([M_TILE, ow], f32, tag="ot",
                                   name=f"ot_{_it}_{m}_{base}")
                    if "sqrt" in _skip:
                        nc.vector.tensor_copy(out=ot, in_=psm)
                    else:
                        nc.scalar.activation(out=ot, in_=psm, func=Act.Sqrt,
                                             bias=x2cols[:, m:m + 1],
                                             scale=-2.0)
                    if "out_dma" in _skip:
                        continue
                    # alternate the two HWDGE rings (SP + ACT paths)
                    dma_eng = nc.sync if m == 0 else nc.scalar
                    dma_eng.dma_start(
                        out=out[m * M_TILE:(m + 1) * M_TILE,
                                base:base + ow],
                        in_=ot)

    nc.compile()
    return nc


def _make_runner(nc):
    """Cached jitted SPMD executor (mirrors bass2jax.run_bass_via_pjrt, but
    reuses one jax.jit wrapper so the NEFF is not re-loaded per call)."""
    import jax
    from jax.experimental.shard_map import shard_map
    from jax.sharding import Mesh, PartitionSpec

    from concourse import bass2jax, mybir

    bass2jax.install_neuronx_cc_hook()
    assert nc.dbg_addr is None

    partition_name = (nc.partition_id_tensor.name
                      if nc.partition_id_tensor else None)
    in_names, out_names, out_avals, zero_shapes = [], [], [], []
    for alloc in nc.m.functions[0].allocations:
        if not isinstance(alloc, mybir.MemoryLocationSet):
            continue
        name = alloc.memorylocations[0].name
        if alloc.kind == "ExternalInput":
            if name != partition_name:
                in_names.append(name)
        elif alloc.kind == "ExternalOutput":
            shape = tuple(alloc.tensor_shape)
            dtype = mybir.dt.np(alloc.dtype)
            out_names.append(name)
            out_avals.append(jax.core.ShapedArray(shape, dtype))
            zero_shapes.append((shape, dtype))
    n_params = len(in_names)
    n_outs = len(out_names)
    all_in_names = list(in_names + out_names)
    if partition_name is not None:
        all_in_names.append(partition_name)
    all_in_names = tuple(all_in_names)
    donate = tuple(range(n_params, n_params + n_outs))

    def _body(*args):
        operands = list(args)
        if partition_name is not None:
            operands.append(bass2jax.partition_id_tensor())
        outs = bass2jax._bass_exec_p.bind(
            *operands,
            out_avals=tuple(out_avals),
            in_names=all_in_names,
            out_names=tuple(out_names),
            lowering_input_output_aliases=(),
            sim_require_finite=True,
            sim_require_nnan=True,
            nc=nc,
        )
        return tuple(outs)

    devices = jax.devices()[:N_CORES]
    mesh = Mesh(np.asarray(devices), ("core",))
    sharded = jax.jit(
        shard_map(_body, mesh=mesh,
                  in_specs=(PartitionSpec("core"),) * (n_params + n_outs),
                  out_specs=(PartitionSpec("core"),) * n_outs,
                  check_rep=False),
        donate_argnums=donate, keep_unused=True)

    def run(in_maps):
        concat_in = [
            np.concatenate([np.asarray(m[name]) for m in in_maps], axis=0)
            for name in in_names
        ]
        concat_zeros = [
            np.zeros((N_CORES * s[0], *s[1:]), dt) for s, dt in zero_shapes
        ]
        out_arrs = sharded(*concat_in, *concat_zeros)
        return [
            {name: np.asarray(out_arrs[i]).reshape(N_CORES, *zero_shapes[i][0])[c]
             for i, name in enumerate(out_names)}
            for c in range(N_CORES)
        ]

    return run


def _get_runner():
    if "run" not in _cache:
        _cache["run"] = _make_runner(_build_nc())
    return _cache["run"]


def kernel(x, y, **_ignored):
    x = np.ascontiguousarray(np.asarray(x), dtype=np.float32)
    y = np.ascontiguousarray(np.asarray(y), dtype=np.float32)
    assert x.shape == (N, D) and y.shape == (N, D)

    run = _get_runner()
    in_maps = [
        {"xs": x[c * ROWS_PER_CORE:(c + 1) * ROWS_PER_CORE, :], "y": y}
        for c in range(N_CORES)
    ]
    results = run(in_maps)
    return np.concatenate([results[c]["out"] for c in range(N_CORES)], axis=0)
